# revision 5
# baseline (speedup 1.0000x reference)
"""DAS (delay-and-sum) beamforming kernel for Trainium2, 8 NeuronCores.

out[b, z, x, k] = sum_nc( (1-w)*rfs[b,k,nc,i0] + w*rfs[b,k,nc,i0+1] ),
idx = samples_idx[ids[b], nc, z, x], i0 = floor(idx), w = idx - i0.

Strategy (pixel sharding): 65536 pixels / 8 cores = 8192 per core; rfs
replicated. Per core, 16 passes over the 128 (b,nc) pairs (8 per pass).

  - SBUF table per pass (host pre-interleaved; ships bf16 and widens
    to fp32 on the idle ACT engine one pass ahead -- pass 0 ships fp32
    so the first gather never waits; halves table transfer):
      partition 16g+k   = rfs[b,k,nc,:]            (v0 rows)
      partition 16g+8+k = rfs[b,k,nc,1:] ++ [0]    (v1 rows, shifted)
    One GPSIMD ap_gather with host-computed int16 floor indices (wrapped
    across the 16 partitions of each group) fetches v0=S[i0], v1=S[i0+1]
    for all 8 k at once.
  - Weights arrive precomputed from host as fp16 rows ((1-w) for v0
    lanes, w for v1 lanes), UNREPLICATED in DRAM; a 0-stride broadcast
    DMA fans each row out to its 8 k partitions. One DVE multiply makes
    P = G*F in fp16; PE then reduces over all 128 lanes with an all-ones
    stationary k-selector: psum[8c%32*4.., q] += sum_p P[p, 512c+q] *
    sel[p, k], accumulated over the 8 passes of each b. Chunk c sits in
    psum bank c//4 at PE col-tile position 32*(c%4).
  - Engine budget per core (cost model): gather 186us (bottleneck, 93%
    busy), DMA 130us, DVE mult 144us, PE 102us, ACT widen 33us -- all
    overlapped; total ~199.6us. No floor/frac arithmetic on device at all. Outputs leave as
    full psum-bank images (one DMA per bank, final banks drained via the
    idle DVE/SP queues); the host slices out the 8 valid rows per 32.
"""
import ml_dtypes
import numpy as np

import concourse.bacc as bacc
import concourse.tile as tile
import concourse.mybir as mybir
from concourse.bass_utils import run_bass_kernel_spmd

dt = mybir.dt

B, K, NC, NS = 2, 8, 64, 2048
NZ, NX = 256, 256
NPIX = NZ * NX
NCORES = 8
SH = NPIX // NCORES          # pixels per core = 8192
NPASS = (B * NC) // 8        # 16 passes, 8 (b,nc) groups per pass
CW = SH // 16                # wrapped idx columns per pass = 512
CHUNK = 512                  # pixels per matmul (psum free dim)
NCHUNK = SH // CHUNK         # 16

_CACHE = {}


def _build_program():
    nc = bacc.Bacc("TRN2", target_bir_lowering=False, debug=False)
    # host-interleaved v0/v1 table (slot = 16g + 8t + k): costs 2x the
    # transfer of a compact layout, but loads in ONE DMA per pass -- DMA
    # instruction count (HWDGE ~0.64us each, shared) beats bytes here
    tab_d = nc.dram_tensor("tab", [NPASS, 128, NS], dt.bfloat16,
                           kind="ExternalInput")
    tab0_d = nc.dram_tensor("tab0", [128, NS], dt.float32,
                            kind="ExternalInput")
    idx_d = nc.dram_tensor("idx", [128, NPASS * CW], dt.int16,
                           kind="ExternalInput")
    fw_d = nc.dram_tensor("fw", [NPASS, 8, 2, SH], dt.float16,
                          kind="ExternalInput")
    sel_d = nc.dram_tensor("sel", [128, K], dt.float16, kind="ExternalInput")
    # full psum-bank images (valid rows 32s..32s+7); host slices. One big
    # DMA per bank beats 4 small ones: HWDGE dispatch ~0.63us each.
    out_d = nc.dram_tensor("out", [B, 4, 128, CHUNK], dt.float32,
                           kind="ExternalOutput")

    with tile.TileContext(nc) as tc:
        from contextlib import ExitStack
        with ExitStack() as ctx:
            tp = ctx.enter_context(tc.tile_pool(name="tabs", bufs=2))
            gp = ctx.enter_context(tc.tile_pool(name="gath", bufs=2))
            fp = ctx.enter_context(tc.tile_pool(name="frac", bufs=2))
            qp = ctx.enter_context(tc.tile_pool(name="prod", bufs=2))
            sp = ctx.enter_context(tc.tile_pool(name="small", bufs=1))
            hp = ctx.enter_context(tc.tile_pool(name="half", bufs=2))
            pp = ctx.enter_context(tc.tile_pool(name="ps", bufs=1, space="PSUM"))

            sel_t = sp.tile([128, K], dt.float16, name="sel_t")
            idx_t = sp.tile([128, NPASS * CW], dt.int16, name="idx_t")

            # 4 psum banks per b; chunk c lives in bank c//4 at partition
            # base 32*(c%4) (PE col-tile positions), rows base..base+7.
            psums = [
                [
                    pp.tile([128, CHUNK], dt.float32, tag=f"ps{b}_{tc}",
                            name=f"ps{b}_{tc}")
                    for tc in range(4)
                ]
                for b in range(B)
            ]
            # memset once: marks the never-matmul'd gap partitions valid so
            # the tail copies can move whole tiles in one op each
            for b in range(B):
                for tc in range(4):
                    nc.vector.memset(psums[b][tc][:, :], 0.0)

            for p in range(NPASS):
                b = p // 8

                # table: pass 0 ships fp32 (direct, first gather's long
                # pole); later passes ship bf16 (half the transfer) and
                # are widened to fp32 on the idle ACT engine, prefetched
                # during the PREVIOUS pass so the widen never gates a
                # gather. Ts[p] was created in iteration p-1.
                if p == 0:
                    T = tp.tile([128, NS], dt.float32, tag="T")
                    nc.sync.dma_start(T[:, :], tab0_d[:, :])
                else:
                    T = Tnext
                nc.sync.dma_start(
                    idx_t[:, p * CW:(p + 1) * CW],
                    idx_d[:, p * CW:(p + 1) * CW],
                )
                ix = idx_t[:, p * CW:(p + 1) * CW]
                if p + 1 < NPASS:
                    Th = hp.tile([128, NS], dt.bfloat16, tag="Th")
                    nc.sync.dma_start(Th[:, :], tab_d[p + 1, :, :])
                    Tnext = tp.tile([128, NS], dt.float32, tag="T")
                    nc.scalar.copy(Tnext[:, :], Th[:, :])

                # (1-w)|w rows, broadcast each row to its 8 k partitions:
                # F[16g+8t+k] = fw[p, g, t] (trailing 0-stride merges to a
                # 3-dim AP, so this is a single DMA)
                F = fp.tile([128, SH], dt.float16, tag="F")
                src = fw_d[p].unsqueeze(2).broadcast_to([8, 2, 8, SH])
                nc.sync.dma_start(F[:, :], src)
                if p == 0:
                    # sel is tiny and first needed by the first matmul
                    # (~t+14us); ACT queue, after the critical loads
                    nc.scalar.dma_start(sel_t[:, :], sel_d[:, :])

                # split each pass so DVE/PE trail the gather by a fraction
                # of a pass; the closing pass of each b splits by 4 so each
                # quarter finishes exactly one psum bank, whose copy-out
                # then overlaps the next quarter.
                last = p % 8 == 7
                NSP = 4 if last else 2
                W = SH // NSP
                CWS = CW // NSP
                CPS = NCHUNK // NSP
                for h in range(NSP):
                    Gt = gp.tile([128, SH // 2], dt.float32, tag=f"G{h % 2}")
                    G = Gt[:, :W]
                    nc.gpsimd.ap_gather(
                        G.rearrange("p (n i) -> p n i", i=1),
                        T[:, :].rearrange("p (n i) -> p n i", i=1),
                        ix[:, h * CWS:(h + 1) * CWS],
                        channels=128,
                        num_elems=NS,
                        d=1,
                        num_idxs=W,
                    )

                    Pt = qp.tile([128, SH // 2], dt.float16, tag=f"P{h % 2}")
                    P = Pt[:, :W]
                    nc.vector.tensor_mul(P, G, F[:, h * W:(h + 1) * W])

                    for cc in range(CPS):
                        c = h * CPS + cc
                        tc, pos = c // 4, 32 * (c % 4)
                        nc.tensor.matmul(
                            psums[b][tc][pos:pos + K, :],
                            sel_t[:, :],
                            P[:, cc * CHUNK:(cc + 1) * CHUNK],
                            start=(p % 8 == 0),
                            stop=last,
                            skip_group_check=True,
                            tile_position=(0, pos),
                        )

                    if last:
                        # quarter h completed psum bank h: drain it now.
                        # b=0 drains go on the ACT queue (SP is busy with
                        # the next passes' loads); for the final b, tiles
                        # 2-3's out DMAs go on the now-idle SP queue --
                        # one queue's serial ~0.66us/dispatch would
                        # otherwise dominate the kernel tail.
                        tc = h
                        cp = sp.tile([128, CHUNK], dt.float32,
                                     tag=f"cp{b}_{tc}", name=f"cp{b}_{tc}")
                        final = b == B - 1 and h >= 2
                        if final:
                            # idle engines at the very end: DVE copies,
                            # SP queue for the out DMAs
                            nc.vector.tensor_copy(cp[:, :], psums[b][tc][:, :])
                        else:
                            nc.scalar.copy(cp[:, :], psums[b][tc][:, :])
                        eng = nc.sync if final else nc.scalar
                        eng.dma_start(out_d[b, tc], cp[:, :])

    nc.compile()
    return nc


def _host_prep(rfs, ids, samples_idx):
    rfs = np.asarray(rfs, dtype=np.float32)
    ids = np.asarray(ids).astype(np.int64)
    samples_idx = np.asarray(samples_idx, dtype=np.float32)

    # table rows: tab[p, 16g+8t+k] = rfs[b, k, 8*(p%8)+g, :], t=1 shifted
    s_rows = rfs.transpose(0, 2, 1, 3)                   # b, nc, k, s
    sh_rows = np.zeros_like(s_rows)
    sh_rows[..., : NS - 1] = s_rows[..., 1:]
    both = np.stack([s_rows, sh_rows], axis=2)           # b, nc, t, k, s
    tabf = both.reshape(NPASS, 128, NS)
    # bulk table ships bf16 (device widens on ACT); pass 0 ships fp32 so
    # the first gather never waits on a widen
    tab = np.ascontiguousarray(tabf.astype(ml_dtypes.bfloat16))
    tab0 = np.ascontiguousarray(tabf[0].astype(np.float32))

    idx = samples_idx[ids].reshape(B, NC, NPIX)          # [2, 64, 65536]
    i0_all = np.floor(idx)
    w_all = (idx - i0_all).astype(np.float16)
    omw_all = (1.0 - w_all.astype(np.float32)).astype(np.float16)
    i0_all = i0_all.astype(np.int16)

    # sel[16g+8t+k', k] = (k'==k), both taps
    sel = np.zeros((128, K), dtype=np.float16)
    slots = np.arange(128)
    sel[slots, slots % 8] = 1.0

    in_maps = []
    for c in range(NCORES):
        lo, hi = c * SH, (c + 1) * SH
        i0 = i0_all[:, :, lo:hi]                         # [B, NC, SH] i16
        # wrapped: partition 16g+m, free (pass, col); pixel q = 16*col + m
        t = i0.reshape(B, 8, 8, CW, 16)                  # b, ncg, g, col, m
        t = t.transpose(2, 4, 0, 1, 3)                   # g, m, b, ncg, col
        idxw = np.ascontiguousarray(t.reshape(128, NPASS * CW))
        # fw[p, g, t, q]: t=0 -> (1-w), t=1 -> w, for (b,nc) of (p,g)
        pair = np.stack(
            [omw_all[:, :, lo:hi], w_all[:, :, lo:hi]], axis=2
        )                                                # b, nc, t, q
        fw = np.ascontiguousarray(
            pair.reshape(B, 8, 8, 2, SH).reshape(NPASS, 8, 2, SH)
        )
        in_maps.append(dict(tab=tab, tab0=tab0, idx=idxw, fw=fw, sel=sel))
    return in_maps


def kernel(rfs, ids, samples_idx):
    if "nc" not in _CACHE:
        _CACHE["nc"] = _build_program()
    nc = _CACHE["nc"]

    in_maps = _host_prep(rfs, ids, samples_idx)
    res = run_bass_kernel_spmd(nc, in_maps, core_ids=list(range(NCORES)))

    out = np.empty((B, NPIX, K), dtype=np.float32)
    for c in range(NCORES):
        o = res.results[c]["out"]                        # [B, 4, 128, 512]
        o = o.reshape(B, 4, 4, 32, CHUNK)[:, :, :, :K, :]  # b, tc, s, k, q
        o = o.transpose(0, 1, 2, 4, 3)                   # b, tc, s, q, k
        out[:, c * SH:(c + 1) * SH, :] = o.reshape(B, SH, K)
    return out.reshape(B, NZ, NX, K)


# revision 6
# speedup vs baseline: 1.0018x; 1.0018x over previous
"""DAS (delay-and-sum) beamforming kernel for Trainium2, 8 NeuronCores.

out[b, z, x, k] = sum_nc( (1-w)*rfs[b,k,nc,i0] + w*rfs[b,k,nc,i0+1] ),
idx = samples_idx[ids[b], nc, z, x], i0 = floor(idx), w = idx - i0.

Strategy (pixel sharding): 65536 pixels / 8 cores = 8192 per core; rfs
replicated. Per core, 16 passes over the 128 (b,nc) pairs (8 per pass).

  - SBUF table per pass (host pre-interleaved; ships bf16 and widens
    to fp32 on the idle ACT engine one pass ahead -- pass 0 ships fp32
    so the first gather never waits; halves table transfer):
      partition 16g+k   = rfs[b,k,nc,:]            (v0 rows)
      partition 16g+8+k = rfs[b,k,nc,1:] ++ [0]    (v1 rows, shifted)
    One GPSIMD ap_gather with host-computed int16 floor indices (wrapped
    across the 16 partitions of each group) fetches v0=S[i0], v1=S[i0+1]
    for all 8 k at once.
  - Weights arrive precomputed from host as fp16 rows ((1-w) for v0
    lanes, w for v1 lanes), UNREPLICATED in DRAM; a 0-stride broadcast
    DMA fans each row out to its 8 k partitions. One DVE multiply makes
    P = G*F in fp16; PE then reduces over all 128 lanes with an all-ones
    stationary k-selector: psum[8c%32*4.., q] += sum_p P[p, 512c+q] *
    sel[p, k], accumulated over the 8 passes of each b. Chunk c sits in
    psum bank c//4 at PE col-tile position 32*(c%4).
  - Engine budget per core (cost model): gather 186us (bottleneck, 93%
    busy), DMA 130us, DVE mult 144us, PE 102us, ACT widen 33us -- all
    overlapped; total ~199.6us. No floor/frac arithmetic on device at all. Outputs leave as
    full psum-bank images (one DMA per bank, final banks drained via the
    idle DVE/SP queues); the host slices out the 8 valid rows per 32.
"""
import ml_dtypes
import numpy as np

import concourse.bacc as bacc
import concourse.tile as tile
import concourse.mybir as mybir
from concourse.bass_utils import run_bass_kernel_spmd

dt = mybir.dt

B, K, NC, NS = 2, 8, 64, 2048
NZ, NX = 256, 256
NPIX = NZ * NX
NCORES = 8
SH = NPIX // NCORES          # pixels per core = 8192
NPASS = (B * NC) // 8        # 16 passes, 8 (b,nc) groups per pass
CW = SH // 16                # wrapped idx columns per pass = 512
CHUNK = 512                  # pixels per matmul (psum free dim)
NCHUNK = SH // CHUNK         # 16

_CACHE = {}


def _build_program():
    nc = bacc.Bacc("TRN2", target_bir_lowering=False, debug=False)
    # host-interleaved v0/v1 table (slot = 16g + 8t + k): costs 2x the
    # transfer of a compact layout, but loads in ONE DMA per pass -- DMA
    # instruction count (HWDGE ~0.64us each, shared) beats bytes here
    tab_d = nc.dram_tensor("tab", [NPASS, 128, NS], dt.bfloat16,
                           kind="ExternalInput")
    tab0_d = nc.dram_tensor("tab0", [128, NS], dt.float32,
                            kind="ExternalInput")
    idx_d = nc.dram_tensor("idx", [128, NPASS * CW], dt.int16,
                           kind="ExternalInput")
    fw_d = nc.dram_tensor("fw", [NPASS, 8, 2, SH], dt.float16,
                          kind="ExternalInput")
    sel_d = nc.dram_tensor("sel", [128, K], dt.float16, kind="ExternalInput")
    # full psum-bank images (valid rows 32s..32s+7); host slices. One big
    # DMA per bank beats 4 small ones: HWDGE dispatch ~0.63us each.
    out_d = nc.dram_tensor("out", [B, 4, 128, CHUNK], dt.float16,
                           kind="ExternalOutput")

    with tile.TileContext(nc) as tc:
        from contextlib import ExitStack
        with ExitStack() as ctx:
            tp = ctx.enter_context(tc.tile_pool(name="tabs", bufs=2))
            gp = ctx.enter_context(tc.tile_pool(name="gath", bufs=2))
            fp = ctx.enter_context(tc.tile_pool(name="frac", bufs=2))
            qp = ctx.enter_context(tc.tile_pool(name="prod", bufs=2))
            sp = ctx.enter_context(tc.tile_pool(name="small", bufs=1))
            hp = ctx.enter_context(tc.tile_pool(name="half", bufs=2))
            pp = ctx.enter_context(tc.tile_pool(name="ps", bufs=1, space="PSUM"))

            sel_t = sp.tile([128, K], dt.float16, name="sel_t")
            idx_t = sp.tile([128, NPASS * CW], dt.int16, name="idx_t")

            # 4 psum banks per b; chunk c lives in bank c//4 at partition
            # base 32*(c%4) (PE col-tile positions), rows base..base+7.
            psums = [
                [
                    pp.tile([128, CHUNK], dt.float32, tag=f"ps{b}_{tc}",
                            name=f"ps{b}_{tc}")
                    for tc in range(4)
                ]
                for b in range(B)
            ]
            # memset once: marks the never-matmul'd gap partitions valid so
            # the tail copies can move whole tiles in one op each
            for b in range(B):
                for tc in range(4):
                    nc.vector.memset(psums[b][tc][:, :], 0.0)

            for p in range(NPASS):
                b = p // 8

                # table: pass 0 ships fp32 (direct, first gather's long
                # pole); later passes ship bf16 (half the transfer) and
                # are widened to fp32 on the idle ACT engine, prefetched
                # during the PREVIOUS pass so the widen never gates a
                # gather. Ts[p] was created in iteration p-1.
                if p == 0:
                    T = tp.tile([128, NS], dt.float32, tag="T")
                    nc.sync.dma_start(T[:, :], tab0_d[:, :])
                else:
                    T = Tnext
                nc.sync.dma_start(
                    idx_t[:, p * CW:(p + 1) * CW],
                    idx_d[:, p * CW:(p + 1) * CW],
                )
                ix = idx_t[:, p * CW:(p + 1) * CW]

                # (1-w)|w rows, broadcast each row to its 8 k partitions:
                # F[16g+8t+k] = fw[p, g, t] (trailing 0-stride merges to a
                # 3-dim AP, so this is a single DMA)
                F = fp.tile([128, SH], dt.float16, tag="F")
                src = fw_d[p].unsqueeze(2).broadcast_to([8, 2, 8, SH])
                nc.sync.dma_start(F[:, :], src)
                if p + 1 < NPASS:
                    # prefetch+widen next pass's table AFTER this pass's
                    # F load (F gates this pass's mult; Th only gates the
                    # NEXT pass's gather)
                    Th = hp.tile([128, NS], dt.bfloat16, tag="Th")
                    nc.sync.dma_start(Th[:, :], tab_d[p + 1, :, :])
                    Tnext = tp.tile([128, NS], dt.float32, tag="T")
                    nc.scalar.copy(Tnext[:, :], Th[:, :])
                if p == 0:
                    # sel is tiny and first needed by the first matmul
                    # (~t+14us); ACT queue, after the critical loads
                    nc.scalar.dma_start(sel_t[:, :], sel_d[:, :])

                # split each pass so DVE/PE trail the gather by a fraction
                # of a pass; the closing pass of each b splits by 4 so each
                # quarter finishes exactly one psum bank, whose copy-out
                # then overlaps the next quarter.
                last = p % 8 == 7
                NSP = 4 if last else 2
                W = SH // NSP
                CWS = CW // NSP
                CPS = NCHUNK // NSP
                for h in range(NSP):
                    Gt = gp.tile([128, SH // 2], dt.float32, tag=f"G{h % 2}")
                    G = Gt[:, :W]
                    nc.gpsimd.ap_gather(
                        G.rearrange("p (n i) -> p n i", i=1),
                        T[:, :].rearrange("p (n i) -> p n i", i=1),
                        ix[:, h * CWS:(h + 1) * CWS],
                        channels=128,
                        num_elems=NS,
                        d=1,
                        num_idxs=W,
                    )

                    Pt = qp.tile([128, SH // 2], dt.float16, tag=f"P{h % 2}")
                    P = Pt[:, :W]
                    nc.vector.tensor_mul(P, G, F[:, h * W:(h + 1) * W])

                    for cc in range(CPS):
                        c = h * CPS + cc
                        tc, pos = c // 4, 32 * (c % 4)
                        nc.tensor.matmul(
                            psums[b][tc][pos:pos + K, :],
                            sel_t[:, :],
                            P[:, cc * CHUNK:(cc + 1) * CHUNK],
                            start=(p % 8 == 0),
                            stop=last,
                            skip_group_check=True,
                            tile_position=(0, pos),
                        )

                    if last:
                        # quarter h completed psum bank h: drain it now.
                        # b=0 drains go on the ACT queue (SP is busy with
                        # the next passes' loads); for the final b, tiles
                        # 2-3's out DMAs go on the now-idle SP queue --
                        # one queue's serial ~0.66us/dispatch would
                        # otherwise dominate the kernel tail.
                        tc = h
                        cp = sp.tile([128, CHUNK], dt.float16,
                                     tag=f"cp{b}_{tc}", name=f"cp{b}_{tc}")
                        final = b == B - 1 and h >= 2
                        if final:
                            # idle engines at the very end: DVE copies,
                            # SP queue for the out DMAs
                            nc.vector.tensor_copy(cp[:, :], psums[b][tc][:, :])
                        else:
                            nc.scalar.copy(cp[:, :], psums[b][tc][:, :])
                        eng = nc.sync if final else nc.scalar
                        eng.dma_start(out_d[b, tc], cp[:, :])

    nc.compile()
    return nc


def _host_prep(rfs, ids, samples_idx):
    rfs = np.asarray(rfs, dtype=np.float32)
    ids = np.asarray(ids).astype(np.int64)
    samples_idx = np.asarray(samples_idx, dtype=np.float32)

    # table rows: tab[p, 16g+8t+k] = rfs[b, k, 8*(p%8)+g, :], t=1 shifted
    s_rows = rfs.transpose(0, 2, 1, 3)                   # b, nc, k, s
    sh_rows = np.zeros_like(s_rows)
    sh_rows[..., : NS - 1] = s_rows[..., 1:]
    both = np.stack([s_rows, sh_rows], axis=2)           # b, nc, t, k, s
    tabf = both.reshape(NPASS, 128, NS)
    # bulk table ships bf16 (device widens on ACT); pass 0 ships fp32 so
    # the first gather never waits on a widen
    tab = np.ascontiguousarray(tabf.astype(ml_dtypes.bfloat16))
    tab0 = np.ascontiguousarray(tabf[0].astype(np.float32))

    idx = samples_idx[ids].reshape(B, NC, NPIX)          # [2, 64, 65536]
    i0_all = np.floor(idx)
    w_all = (idx - i0_all).astype(np.float16)
    omw_all = (1.0 - w_all.astype(np.float32)).astype(np.float16)
    i0_all = i0_all.astype(np.int16)

    # sel[16g+8t+k', k] = (k'==k), both taps
    sel = np.zeros((128, K), dtype=np.float16)
    slots = np.arange(128)
    sel[slots, slots % 8] = 1.0

    in_maps = []
    for c in range(NCORES):
        lo, hi = c * SH, (c + 1) * SH
        i0 = i0_all[:, :, lo:hi]                         # [B, NC, SH] i16
        # wrapped: partition 16g+m, free (pass, col); pixel q = 16*col + m
        t = i0.reshape(B, 8, 8, CW, 16)                  # b, ncg, g, col, m
        t = t.transpose(2, 4, 0, 1, 3)                   # g, m, b, ncg, col
        idxw = np.ascontiguousarray(t.reshape(128, NPASS * CW))
        # fw[p, g, t, q]: t=0 -> (1-w), t=1 -> w, for (b,nc) of (p,g)
        pair = np.stack(
            [omw_all[:, :, lo:hi], w_all[:, :, lo:hi]], axis=2
        )                                                # b, nc, t, q
        fw = np.ascontiguousarray(
            pair.reshape(B, 8, 8, 2, SH).reshape(NPASS, 8, 2, SH)
        )
        in_maps.append(dict(tab=tab, tab0=tab0, idx=idxw, fw=fw, sel=sel))
    return in_maps


def kernel(rfs, ids, samples_idx):
    if "nc" not in _CACHE:
        _CACHE["nc"] = _build_program()
    nc = _CACHE["nc"]

    in_maps = _host_prep(rfs, ids, samples_idx)
    res = run_bass_kernel_spmd(nc, in_maps, core_ids=list(range(NCORES)))

    out = np.empty((B, NPIX, K), dtype=np.float32)
    for c in range(NCORES):
        o = res.results[c]["out"]                        # [B, 4, 128, 512]
        o = o.reshape(B, 4, 4, 32, CHUNK)[:, :, :, :K, :]  # b, tc, s, k, q
        o = o.transpose(0, 1, 2, 4, 3)                   # b, tc, s, q, k
        out[:, c * SH:(c + 1) * SH, :] = o.reshape(B, SH, K)
    return out.reshape(B, NZ, NX, K)
